# revision 28
# baseline (speedup 1.0000x reference)
"""Trainium2 Bass kernel for nn_Attention (S=2048, B=2, D=1024, H=16, C=64).

Tensor-parallel over heads across 8 NeuronCores (2 heads/core). v3:
  - host pre-converts x/rope/weights to bf16 and folds the q/k RMSNorm
    weights into the rope factor tables.
  - RMSNorm rstd comes from a single ACT Rsqrt (reciprocal_sqrt_and_small
    table set, shared with the copy/square fillers); k's rstd and the
    1/sqrt(C) score scale are applied before RoPE via PE-broadcast matmuls
    (K padded to 32 - the ISA rejects 2-partition contractions).
  - phase-2 scores use a 3-deep [128,1024] PSUM ring so the PE never waits
    on the exp reads and stays at its fast p-state; all other PSUM tiles
    are 1-bank so the total fits in 8 banks.
  - phase-2 query blocks are strided (128 tokens per output group); each
    block's A2A (4 total) pipelines behind the next block's compute, and
    each out-projection tile starts as soon as its A2A lands.
"""

import sys

if "/opt/trn_rl_repo" not in sys.path:
    sys.path.insert(0, "/opt/trn_rl_repo")

import numpy as np
import ml_dtypes
import concourse.bass as bass
from concourse import bacc, tile, mybir
from concourse.bass_utils import run_bass_kernel_spmd
from concourse.masks import make_identity

S, B, D, H, C = 2048, 2, 1024, 16, 64
EPS = 1e-6
NCORES = 8
T = S * B                  # 4096 tokens, batch-major: t = b*S + s
LH = H // NCORES           # 2 local heads
LC = LH * C                # 128 local head columns
TCH = 512                  # phase-1 token chunk
NCHUNK = T // TCH          # 8
NBLK = 4                   # phase-2 query blocks (128 tokens/group each)
TOK_OUT = T // NCORES      # 512 output tokens per core

F32 = mybir.dt.float32
F32R = mybir.dt.float32r
BF16 = mybir.dt.bfloat16
AF = mybir.ActivationFunctionType

_CACHE = {}
LAST_RESULTS = None


def _build():
    nc = bacc.Bacc("TRN2", target_bir_lowering=False, debug=False,
                   num_devices=NCORES)
    xT = nc.dram_tensor("xT", [D, T], BF16, kind="ExternalInput")
    wq = nc.dram_tensor("wq", [D, LC], BF16, kind="ExternalInput")
    wk = nc.dram_tensor("wk", [D, LC], BF16, kind="ExternalInput")
    wv = nc.dram_tensor("wv", [D, LC], BF16, kind="ExternalInput")
    wo = nc.dram_tensor("wo", [H * C, D], BF16, kind="ExternalInput")
    ropeAq = nc.dram_tensor("ropeAq", [LC, T], BF16, kind="ExternalInput")
    ropeBq = nc.dram_tensor("ropeBq", [LC, T], BF16, kind="ExternalInput")
    ropeAk = nc.dram_tensor("ropeAk", [LC, T], BF16, kind="ExternalInput")
    ropeBk = nc.dram_tensor("ropeBk", [LC, T], BF16, kind="ExternalInput")
    # broadcast-weight constants, zero-padded to K=32:
    # rows 0:32 bcq, 32:64 bck (x 1/sqrt(C)), 64:96 bcd0, 96:128 bcd1
    bcw = nc.dram_tensor("bcw", [128, 128], F32, kind="ExternalInput")
    out = nc.dram_tensor("out", [TOK_OUT, D], F32, kind="ExternalOutput")

    xT4 = xT[:, :].rearrange("(a p) t -> p a t", p=128)       # [128, 8, T]
    wq4 = wq[:, :].rearrange("(a p) c -> p a c", p=128)       # [128, 8, LC]
    wk4 = wk[:, :].rearrange("(a p) c -> p a c", p=128)
    wv4 = wv[:, :].rearrange("(a p) c -> p a c", p=128)
    wo4 = wo[:, :].rearrange("(a p) n -> p a n", p=128)       # [128, 8, D]

    with tile.TileContext(nc) as tc:
        with (
            tc.tile_pool(name="singles", bufs=1) as singles,
            tc.tile_pool(name="xtp", bufs=2) as xtp,
            tc.tile_pool(name="ropep", bufs=3) as ropep,
            tc.tile_pool(name="workp", bufs=3) as workp,
            tc.tile_pool(name="expp", bufs=3) as expp,
            tc.tile_pool(name="outp", bufs=2) as outp,
            tc.tile_pool(name="ps_big", bufs=3, space="PSUM") as ps_big,
            tc.tile_pool(name="ps_op", bufs=2, space="PSUM") as ps_op,
            tc.tile_pool(name="dram", bufs=1, space="DRAM") as dram,
        ):
            # ---- constants ----
            identf = singles.tile([128, 128], F32)
            make_identity(nc, identf)
            identb = singles.tile([128, 128], BF16)
            nc.vector.tensor_copy(identb, identf)
            # per-head column selectors for sum-of-squares: [128, 2]
            ones2f = singles.tile([128, 2], F32)
            nc.vector.memset(ones2f, 0.0)
            nc.vector.memset(ones2f[0:64, 0:1], 1.0 / C)
            nc.vector.memset(ones2f[64:128, 1:2], 1.0 / C)
            ones2 = singles.tile([128, 2], F32R)
            nc.vector.tensor_copy(ones2, ones2f)
            # broadcast weights: rows 0:2 select 64-partition blocks,
            # rows 2:32 are zero padding for the K=32 contraction
            bcq = singles.tile([32, 128], BF16)
            nc.gpsimd.dma_start(out=bcq, in_=bcw[0:32, :])
            bck = singles.tile([32, 128], BF16)
            nc.gpsimd.dma_start(out=bck, in_=bcw[32:64, :])
            # zero-padded moving operands for the broadcasts (rows 0:2
            # live); ping-pong halves for the 2-chunk-deep pipeline
            rstd_s = singles.tile([32, 2, 2 * TCH], BF16)
            nc.vector.memset(rstd_s, 0.0)

            # ---- weights ----
            wq_sb = singles.tile([128, 8, LC], BF16)
            wk_sb = singles.tile([128, 8, LC], BF16)
            wv_sb = singles.tile([128, 8, LC], BF16)
            nc.sync.dma_start(out=wq_sb, in_=wq4)
            nc.sync.dma_start(out=wk_sb, in_=wk4)
            nc.sync.dma_start(out=wv_sb, in_=wv4)

            # ---- persistent activations ----
            qT_sb = singles.tile([128, T], BF16)          # [c_local, t]
            kTz0 = singles.tile([128, T], BF16)   # head0 rows 0:64, rest 0
            kTz1 = singles.tile([128, T], BF16)   # head1 rows 64:128, rest 0
            nc.vector.memset(kTz0[64:128, :], 0.0)
            nc.vector.memset(kTz1[0:64, :], 0.0)
            # [t%128, t//128, v0(64)|one|v1(64)|one]
            v_sb = singles.tile([128, T // 128, 2 * (C + 1)], BF16)
            onescol = singles.tile([128, T // 128, 1], F32)
            nc.vector.memset(onescol, 1.0)
            nc.vector.tensor_copy(v_sb[:, :, C:C + 1], onescol)
            nc.vector.tensor_copy(v_sb[:, :, 2 * C + 1:2 * C + 2], onescol)

            # strided view of qT for phase-2 blocks: [128, g, blk, 128]
            qT_v = qT_sb[:, :].rearrange("p (g k c) -> p g k c", g=8, c=128)

            # ================= phase 1: projections + norm + rope =========
            st = {}

            def emit_proj(ch):
                t0 = ch * TCH
                xt = xtp.tile([128, 8, TCH], BF16, tag="xt")
                nc.sync.dma_start(out=xt, in_=xT4[:, :, t0:t0 + TCH])
                raq = ropep.tile([128, TCH], BF16, tag="raq")
                rbq = ropep.tile([128, TCH], BF16, tag="rbq")
                rak = ropep.tile([128, TCH], BF16, tag="rak")
                rbk = ropep.tile([128, TCH], BF16, tag="rbk")
                nc.sync.dma_start(out=raq, in_=ropeAq[:, t0:t0 + TCH])
                nc.sync.dma_start(out=rbq, in_=ropeBq[:, t0:t0 + TCH])
                nc.sync.dma_start(out=rak, in_=ropeAk[:, t0:t0 + TCH])
                nc.sync.dma_start(out=rbk, in_=ropeBk[:, t0:t0 + TCH])

                psqk = ps_big.tile([128, 2 * TCH], F32, tag="mm2",
                                   name=f"psqk_{ch}")
                for a in range(8):
                    nc.tensor.matmul(psqk[:, 0:TCH], wq_sb[:, a, :],
                                     xt[:, a, :], start=(a == 0), stop=(a == 7))
                for a in range(8):
                    nc.tensor.matmul(psqk[:, TCH:2 * TCH], wk_sb[:, a, :],
                                     xt[:, a, :], start=(a == 0), stop=(a == 7))
                # stage raw q/k in SBUF bf16 (engines read one PSUM input)
                qraw = workp.tile([128, TCH], BF16, tag="qraw", bufs=3)
                nc.scalar.copy(qraw, psqk[:, 0:TCH])
                kraw = workp.tile([128, TCH], BF16, tag="kraw", bufs=3)
                nc.scalar.copy(kraw, psqk[:, TCH:2 * TCH])
                sq = workp.tile([128, TCH], F32R, tag="sq", bufs=2)
                nc.vector.tensor_mul(sq, qraw, qraw)
                sqk = workp.tile([128, TCH], F32R, tag="sqk", bufs=2)
                nc.vector.tensor_mul(sqk, kraw, kraw)
                st[ch] = dict(psqk=psqk, sq=sq, sqk=sqk, qraw=qraw,
                              kraw=kraw, raq=raq, rbq=rbq, rak=rak, rbk=rbk,
                              xt=xt)

            def emit_projv(ch):
                s = st[ch]
                psv = ps_op.tile([128, TCH], F32, tag="po", name=f"psv_{ch}")
                for a in range(8):
                    nc.tensor.matmul(psv, wv_sb[:, a, :], s["xt"][:, a, :],
                                     start=(a == 0), stop=(a == 7))
                s["psv"] = psv

            def emit_norm(ch):
                s = st[ch]
                # v copy first so psv's PSUM slot frees promptly
                vt = workp.tile([128, TCH], BF16, tag="vt", bufs=3)
                nc.scalar.copy(vt, s["psv"])
                s["vt"] = vt
                msq = ps_op.tile([2, TCH], F32, tag="po", name=f"msq_{ch}")
                nc.tensor.matmul(msq, ones2, s["sq"], start=True, stop=True)
                msk = ps_op.tile([2, TCH], F32, tag="po", name=f"msk_{ch}")
                nc.tensor.matmul(msk, ones2, s["sqk"], start=True, stop=True)
                # rstd = sqrt(1/(ms/C)): fast-approx recip on DVE reads
                # ms straight from PSUM (1/C folded into ones2; eps dropped -
                # it only guards all-zero rows), then ACT Sqrt writes bf16
                rqi = workp.tile([2, TCH], F32, tag="rqi", bufs=2,
                                 name=f"rqi_{ch}")
                nc.vector.reciprocal_approx_fast(rqi, msq)
                rki = workp.tile([2, TCH], F32, tag="rki", bufs=2,
                                 name=f"rki_{ch}")
                nc.vector.reciprocal_approx_fast(rki, msk)
                nc.scalar.activation(rstd_s[0:2, ch % 2, 0:TCH], rqi,
                                     AF.Sqrt, bias=0.0, scale=1.0)
                nc.scalar.activation(rstd_s[0:2, ch % 2, TCH:2 * TCH], rki,
                                     AF.Sqrt, bias=0.0, scale=1.0)

            def emit_bcast(ch):
                s = st[ch]
                bcq_ps = ps_op.tile([128, TCH], F32, tag="po",
                                    name=f"bcq_{ch}")
                nc.tensor.matmul(bcq_ps, bcq, rstd_s[:, ch % 2, 0:TCH],
                                 start=True, stop=True)
                bck_ps = ps_op.tile([128, TCH], F32, tag="po",
                                    name=f"bck_{ch}")
                nc.tensor.matmul(bck_ps, bck, rstd_s[:, ch % 2, TCH:2 * TCH],
                                 start=True, stop=True)
                s["bcq_ps"], s["bck_ps"] = bcq_ps, bck_ps

            def emit_rope(ch):
                t0 = ch * TCH
                s = st.pop(ch)
                rotq = workp.tile([128, TCH], BF16, tag="rotq")
                rotk = workp.tile([128, TCH], BF16, tag="rotk")
                qraw, kraw = s["qraw"], s["kraw"]
                for g0 in (0, 64):
                    nc.sync.dma_start(out=rotq[g0:g0 + 32, :],
                                      in_=qraw[g0 + 32:g0 + 64, :])
                    nc.sync.dma_start(out=rotq[g0 + 32:g0 + 64, :],
                                      in_=qraw[g0:g0 + 32, :])
                    nc.sync.dma_start(out=rotk[g0:g0 + 32, :],
                                      in_=kraw[g0 + 32:g0 + 64, :])
                    nc.sync.dma_start(out=rotk[g0 + 32:g0 + 64, :],
                                      in_=kraw[g0:g0 + 32, :])
                t1q = workp.tile([128, TCH], BF16, tag="t1q")
                nc.vector.tensor_mul(t1q, s["raq"], qraw)
                t2q = workp.tile([128, TCH], BF16, tag="t2q")
                nc.gpsimd.tensor_mul(t2q, s["rbq"], rotq)
                qfu = workp.tile([128, TCH], BF16, tag="qfu")
                nc.vector.tensor_add(qfu, t1q, t2q)
                nc.vector.tensor_mul(qT_sb[:, t0:t0 + TCH], qfu, s["bcq_ps"])
                t1k = workp.tile([128, TCH], BF16, tag="t1k")
                nc.vector.tensor_mul(t1k, s["rak"], kraw)
                t2k = workp.tile([128, TCH], BF16, tag="t2k")
                nc.gpsimd.tensor_mul(t2k, s["rbk"], rotk)
                kf = workp.tile([128, TCH], BF16, tag="kf")
                nc.vector.tensor_add(kf, t1k, t2k)
                nc.vector.tensor_mul(kTz0[0:64, t0:t0 + TCH], kf[0:64, :],
                                     s["bck_ps"][0:64, :])
                nc.vector.tensor_mul(kTz1[64:128, t0:t0 + TCH],
                                     kf[64:128, :], s["bck_ps"][64:128, :])
                # v: bf16 transpose into v_sb
                pt = ps_op.tile([128, TCH // 128, 128], BF16, tag="po",
                                name=f"pt_{ch}")
                for s5 in range(TCH // 128):
                    nc.tensor.transpose(pt[:, s5, :],
                                        s["vt"][:, s5 * 128:(s5 + 1) * 128],
                                        identb)
                for s5 in range(TCH // 128):
                    tt = (t0 + s5 * 128) // 128
                    nc.scalar.copy(v_sb[:, tt, 0:C], pt[:, s5, 0:C])
                    nc.scalar.copy(v_sb[:, tt, C + 1:2 * C + 1],
                                   pt[:, s5, C:2 * C])

            emit_proj(0)
            emit_projv(0)
            emit_norm(0)
            emit_proj(1)
            emit_projv(1)
            for ch in range(2, NCHUNK):
                emit_norm(ch - 1)
                emit_proj(ch)
                emit_bcast(ch - 2)
                emit_projv(ch)
                emit_rope(ch - 2)
            emit_norm(NCHUNK - 1)
            for ch in (NCHUNK - 2, NCHUNK - 1):
                emit_bcast(ch)
                emit_rope(ch)

            wo_sb = singles.tile([128, 8, D], BF16)
            nc.sync.dma_start(out=wo_sb, in_=wo4)
            # dummy exp: pulls the ACT exp table load into the phase-1 tail
            dummy = singles.tile([2, 1], F32)
            nc.scalar.activation(dummy, ones2f[0:2, 0:1], AF.Exp, bias=0.0,
                                 scale=1.0)

            # ================= phase 2: attention + pipelined A2A =========
            bounce_in = [dram.tile([NCORES * 128, 128], BF16,
                                   name=f"bnc_in_{k}") for k in range(NBLK)]
            bounce_out = [dram.tile([NCORES * 128, 128], BF16,
                                    name=f"bnc_out_{k}") for k in range(NBLK)]

            def emit_batch_compute(blk, b):
                """scores+exp+av for (block, batch); stage po/den in SBUF"""
                pos = [ps_op.tile([C + 1, TCH], F32, tag="po",
                                  name=f"pos_{blk}_{b}_{lh}")
                       for lh in range(LH)]
                qv = qT_v[:, 4 * b:4 * b + 4, blk, :]
                pend = []

                def emit_scores(jt):
                    j0 = b * S + jt * 128
                    pss = ps_big.tile([128, 2 * TCH], F32, tag="mm2",
                                      name=f"pss_{blk}_{b}_{jt}")
                    nc.tensor.matmul(pss[:, 0:TCH], kTz0[:, j0:j0 + 128], qv,
                                     start=True, stop=True)
                    nc.tensor.matmul(pss[:, TCH:2 * TCH], kTz1[:, j0:j0 + 128],
                                     qv, start=True, stop=True)
                    ex = expp.tile([128, 2 * TCH], BF16, tag="ex",
                                   name=f"ex_{blk}_{b}_{jt}")
                    nc.scalar.activation(ex, pss, AF.Exp, bias=0.0, scale=1.0)
                    return ex

                def emit_av(jt, ex):
                    j0 = b * S + jt * 128
                    for lh in range(LH):
                        nc.tensor.matmul(
                            pos[lh],
                            v_sb[:, j0 // 128,
                                 lh * (C + 1):(lh + 1) * (C + 1)],
                            ex[:, lh * TCH:(lh + 1) * TCH],
                            start=(jt == 0), stop=(jt == S // 128 - 1))

                for jt in range(S // 128):
                    pend.append((jt, emit_scores(jt)))
                    if len(pend) > 2:
                        pj, pex = pend.pop(0)
                        emit_av(pj, pex)
                while pend:
                    pj, pex = pend.pop(0)
                    emit_av(pj, pex)

                # stage po in SBUF (frees the pos PSUM slots) + 1/den
                po_sb = [workp.tile([C + 1, TCH], F32, tag=f"posb{lh}",
                                    bufs=2, name=f"posb_{blk}_{b}_{lh}")
                         for lh in range(LH)]
                for lh in range(LH):
                    nc.vector.tensor_copy(po_sb[lh], pos[lh])
                den2 = workp.tile([2, TCH], F32, tag="den2", bufs=2,
                                  name=f"den2_{blk}_{b}")
                for lh in range(LH):
                    nc.gpsimd.dma_start(out=den2[lh:lh + 1, :],
                                        in_=po_sb[lh][C:C + 1, :])
                rdf = workp.tile([2, TCH], F32, tag="rdf", bufs=2,
                                 name=f"rdf_{blk}_{b}")
                nc.vector.reciprocal_approx_fast(rdf, den2)
                # broadcast 1/den to 64 partitions via a DRAM round trip
                # (keeps the whole normalize chain off the PE queue)
                rdf_dr = dram.tile([2, TCH], F32, tag="rdf_dr", bufs=2,
                                   name=f"rdfdr_{blk}_{b}")
                nc.gpsimd.dma_start(out=rdf_dr, in_=rdf)
                atts = []
                for lh in range(LH):
                    nrm_sb = workp.tile([C, TCH], F32, tag=f"nrmsb{lh}",
                                        bufs=2, name=f"nrmsb_{blk}_{b}_{lh}")
                    nc.gpsimd.dma_start(
                        out=nrm_sb,
                        in_=bass.AP(tensor=rdf_dr.tensor, offset=rdf_dr.offset
                                    + lh * TCH, ap=[[0, C], [1, TCH]]))
                    att = workp.tile([C, TCH], BF16, tag=f"att{lh}", bufs=2,
                                     name=f"att_{blk}_{b}_{lh}")
                    nc.vector.tensor_mul(att, po_sb[lh][0:C, :], nrm_sb)
                    atts.append(att)
                # scatter: columns are 4 groups x 128 tokens (batch b)
                for gg in range(4):
                    g = 4 * b + gg
                    for lh in range(LH):
                        nc.gpsimd.dma_start(
                            out=bounce_in[blk][g * 128 + lh * C:
                                               g * 128 + (lh + 1) * C, :],
                            in_=atts[lh][:, gg * 128:(gg + 1) * 128])

            def emit_block(blk):
                emit_batch_compute(blk, 0)
                emit_batch_compute(blk, 1)
                nc.gpsimd.collective_compute(
                    "AllToAll", mybir.AluOpType.bypass,
                    replica_groups=[list(range(NCORES))],
                    ins=[bounce_in[blk][:, :].opt()],
                    outs=[bounce_out[blk][:, :].opt()])

            def emit_outproj(blk):
                att_all = outp.tile([128, 8, 128], BF16, tag="aat")
                nc.sync.dma_start(
                    out=att_all,
                    in_=bounce_out[blk][:, :].rearrange("(a p) t -> p a t",
                                                        p=128))
                pp = ps_big.tile([128, D], F32, tag="mm2", name=f"pp_{blk}")
                for nh in range(2):
                    for a in range(8):
                        nc.tensor.matmul(
                            pp[:, nh * 512:(nh + 1) * 512], att_all[:, a, :],
                            wo_sb[:, a, nh * 512:(nh + 1) * 512],
                            start=(a == 0), stop=(a == 7))
                out_sb = outp.tile([128, D], F32, tag="osb")
                nc.vector.tensor_copy(out_sb, pp)
                nc.sync.dma_start(out=out[blk * 128:(blk + 1) * 128, :],
                                  in_=out_sb)

            emit_block(0)
            emit_block(1)
            emit_block(2)
            emit_outproj(0)
            emit_block(3)
            emit_outproj(1)
            emit_outproj(2)
            emit_outproj(3)

    nc.compile()
    return nc


def kernel(x, rope_emb, Wq, Wk, Wv, q_norm_w, k_norm_w, Wout):
    global LAST_RESULTS
    if "nc" not in _CACHE:
        _CACHE["nc"] = _build()
    nc = _CACHE["nc"]

    bf = ml_dtypes.bfloat16
    # batch-major tokens: t = b*S + s
    x2 = np.ascontiguousarray(
        np.transpose(np.asarray(x, np.float32), (1, 0, 2)).reshape(T, D))
    xT_np = np.ascontiguousarray(x2.T).astype(bf)

    re = np.asarray(rope_emb, np.float32)
    cosT = np.ascontiguousarray(re[:, :, 0, 0].T)    # [32, S]
    r01T = np.ascontiguousarray(re[:, :, 0, 1].T)
    r10T = np.ascontiguousarray(re[:, :, 1, 0].T)
    cos2 = np.concatenate([cosT, cosT], axis=1)      # [32, T] batch-major
    r01_2 = np.concatenate([r01T, r01T], axis=1)
    r10_2 = np.concatenate([r10T, r10T], axis=1)
    ropeA = np.concatenate([cos2, cos2, cos2, cos2], axis=0)   # [128, T]
    ropeB = np.concatenate([r01_2, r10_2, r01_2, r10_2], axis=0)

    qw_np = np.asarray(q_norm_w, np.float32)
    kw_np = np.asarray(k_norm_w, np.float32)
    wpat_q = np.concatenate([qw_np, qw_np])          # [128]
    wpat_k = np.concatenate([kw_np, kw_np])
    # partner row: swap 32-blocks within each 64-row head group
    pidx = np.arange(128)
    pidx = np.where((pidx % 64) < 32, pidx + 32, pidx - 32)
    ropeAq_np = (ropeA * wpat_q[:, None]).astype(bf)
    ropeBq_np = (ropeB * wpat_q[pidx][:, None]).astype(bf)
    ropeAk_np = (ropeA * wpat_k[:, None]).astype(bf)
    ropeBk_np = (ropeB * wpat_k[pidx][:, None]).astype(bf)

    Wq = np.asarray(Wq, np.float32)
    Wk = np.asarray(Wk, np.float32)
    Wv = np.asarray(Wv, np.float32)
    Wout = np.ascontiguousarray(np.asarray(Wout, np.float32)).astype(bf)

    bcw_np = np.zeros((128, 128), np.float32)
    bcw_np[0, 0:64] = 1.0                      # bcq
    bcw_np[1, 64:128] = 1.0
    bcw_np[32, 0:64] = 1.0 / np.sqrt(C)        # bck
    bcw_np[33, 64:128] = 1.0 / np.sqrt(C)
    bcw_np[64, 0:64] = 1.0                     # bcd0 row0
    bcw_np[97, 0:64] = 1.0                     # bcd1 row1

    in_maps = []
    for g in range(NCORES):
        sl = slice(g * LC, (g + 1) * LC)
        in_maps.append({
            "xT": xT_np,
            "wq": np.ascontiguousarray(Wq[:, sl]).astype(bf),
            "wk": np.ascontiguousarray(Wk[:, sl]).astype(bf),
            "wv": np.ascontiguousarray(Wv[:, sl]).astype(bf),
            "wo": Wout,
            "ropeAq": ropeAq_np, "ropeBq": ropeBq_np,
            "ropeAk": ropeAk_np, "ropeBk": ropeBk_np,
            "bcw": bcw_np,
        })

    res = run_bass_kernel_spmd(nc, in_maps, core_ids=list(range(NCORES)))
    LAST_RESULTS = res
    # core g returns out rows = its 4 x 128-token tiles: token t = g*512 + r
    out_full = np.concatenate([res.results[g]["out"] for g in range(NCORES)],
                              axis=0)                 # [T, D] batch-major
    return np.ascontiguousarray(
        out_full.reshape(B, S, D).transpose(1, 0, 2).astype(np.float32))


# revision 29
# speedup vs baseline: 1.1480x; 1.1480x over previous
"""Trainium2 Bass kernel for nn_Attention (S=2048, B=2, D=1024, H=16, C=64).

Tensor-parallel over heads across 8 NeuronCores (2 heads/core). v3:
  - host pre-converts x/rope/weights to bf16 and folds the q/k RMSNorm
    weights into the rope factor tables.
  - RMSNorm rstd comes from a single ACT Rsqrt (reciprocal_sqrt_and_small
    table set, shared with the copy/square fillers); k's rstd and the
    1/sqrt(C) score scale are applied before RoPE via PE-broadcast matmuls
    (K padded to 32 - the ISA rejects 2-partition contractions).
  - phase-2 scores use a 3-deep [128,1024] PSUM ring so the PE never waits
    on the exp reads and stays at its fast p-state; all other PSUM tiles
    are 1-bank so the total fits in 8 banks.
  - phase-2 query blocks are strided (128 tokens per output group); each
    block's A2A (4 total) pipelines behind the next block's compute, and
    each out-projection tile starts as soon as its A2A lands.
"""

import sys

if "/opt/trn_rl_repo" not in sys.path:
    sys.path.insert(0, "/opt/trn_rl_repo")

import numpy as np
import ml_dtypes
import concourse.bass as bass
from concourse import bacc, tile, mybir
from concourse.bass_utils import run_bass_kernel_spmd
from concourse.masks import make_identity

S, B, D, H, C = 2048, 2, 1024, 16, 64
EPS = 1e-6
NCORES = 8
T = S * B                  # 4096 tokens, batch-major: t = b*S + s
LH = H // NCORES           # 2 local heads
LC = LH * C                # 128 local head columns
TCH = 512                  # phase-1 token chunk
NCHUNK = T // TCH          # 8
NBLK = 4                   # phase-2 query blocks (128 tokens/group each)
TOK_OUT = T // NCORES      # 512 output tokens per core

F32 = mybir.dt.float32
F32R = mybir.dt.float32r
BF16 = mybir.dt.bfloat16
AF = mybir.ActivationFunctionType

_CACHE = {}
LAST_RESULTS = None


def _build():
    nc = bacc.Bacc("TRN2", target_bir_lowering=False, debug=False,
                   num_devices=NCORES)
    xT = nc.dram_tensor("xT", [D, T], BF16, kind="ExternalInput")
    wq = nc.dram_tensor("wq", [D, LC], BF16, kind="ExternalInput")
    wk = nc.dram_tensor("wk", [D, LC], BF16, kind="ExternalInput")
    wv = nc.dram_tensor("wv", [D, LC], BF16, kind="ExternalInput")
    wo = nc.dram_tensor("wo", [H * C, D], BF16, kind="ExternalInput")
    ropeAq = nc.dram_tensor("ropeAq", [LC, T], BF16, kind="ExternalInput")
    ropeBq = nc.dram_tensor("ropeBq", [LC, T], BF16, kind="ExternalInput")
    ropeAk = nc.dram_tensor("ropeAk", [LC, T], BF16, kind="ExternalInput")
    ropeBk = nc.dram_tensor("ropeBk", [LC, T], BF16, kind="ExternalInput")
    # broadcast-weight constants, zero-padded to K=32:
    # rows 0:32 bcq, 32:64 bck (x 1/sqrt(C)), 64:96 bcd0, 96:128 bcd1
    bcw = nc.dram_tensor("bcw", [128, 128], F32, kind="ExternalInput")
    out = nc.dram_tensor("out", [TOK_OUT, D], F32, kind="ExternalOutput")

    xT4 = xT[:, :].rearrange("(a p) t -> p a t", p=128)       # [128, 8, T]
    wq4 = wq[:, :].rearrange("(a p) c -> p a c", p=128)       # [128, 8, LC]
    wk4 = wk[:, :].rearrange("(a p) c -> p a c", p=128)
    wv4 = wv[:, :].rearrange("(a p) c -> p a c", p=128)
    wo4 = wo[:, :].rearrange("(a p) n -> p a n", p=128)       # [128, 8, D]

    with tile.TileContext(nc) as tc:
        with (
            tc.tile_pool(name="singles", bufs=1) as singles,
            tc.tile_pool(name="xtp", bufs=2) as xtp,
            tc.tile_pool(name="ropep", bufs=3) as ropep,
            tc.tile_pool(name="workp", bufs=3) as workp,
            tc.tile_pool(name="expp", bufs=3) as expp,
            tc.tile_pool(name="outp", bufs=2) as outp,
            tc.tile_pool(name="ps_big", bufs=3, space="PSUM") as ps_big,
            tc.tile_pool(name="ps_op", bufs=2, space="PSUM") as ps_op,
            tc.tile_pool(name="dram", bufs=1, space="DRAM") as dram,
        ):
            # ---- constants ----
            identf = singles.tile([128, 128], F32)
            make_identity(nc, identf)
            identb = singles.tile([128, 128], BF16)
            nc.vector.tensor_copy(identb, identf)
            # per-head column selectors for sum-of-squares: [128, 2]
            ones2f = singles.tile([128, 2], F32)
            nc.vector.memset(ones2f, 0.0)
            nc.vector.memset(ones2f[0:64, 0:1], 1.0 / C)
            nc.vector.memset(ones2f[64:128, 1:2], 1.0 / C)
            ones2 = singles.tile([128, 2], F32R)
            nc.vector.tensor_copy(ones2, ones2f)
            # broadcast weights: rows 0:2 select 64-partition blocks,
            # rows 2:32 are zero padding for the K=32 contraction
            bcq = singles.tile([32, 128], BF16)
            nc.gpsimd.dma_start(out=bcq, in_=bcw[0:32, :])
            bck = singles.tile([32, 128], BF16)
            nc.gpsimd.dma_start(out=bck, in_=bcw[32:64, :])
            # zero-padded moving operands for the broadcasts (rows 0:2
            # live); ping-pong halves for the 2-chunk-deep pipeline
            rstd_s = singles.tile([32, 2, 2 * TCH], BF16)
            nc.vector.memset(rstd_s, 0.0)

            # ---- weights ----
            wq_sb = singles.tile([128, 8, LC], BF16)
            wk_sb = singles.tile([128, 8, LC], BF16)
            wv_sb = singles.tile([128, 8, LC], BF16)
            nc.sync.dma_start(out=wq_sb, in_=wq4)
            nc.sync.dma_start(out=wk_sb, in_=wk4)
            nc.sync.dma_start(out=wv_sb, in_=wv4)

            # ---- persistent activations ----
            qT_sb = singles.tile([128, T], BF16)          # [c_local, t]
            kTz0 = singles.tile([128, T], BF16)   # head0 rows 0:64, rest 0
            kTz1 = singles.tile([128, T], BF16)   # head1 rows 64:128, rest 0
            nc.vector.memset(kTz0[64:128, :], 0.0)
            nc.vector.memset(kTz1[0:64, :], 0.0)
            # [t%128, t//128, v0(64)|one|v1(64)|one]
            v_sb = singles.tile([128, T // 128, 2 * (C + 1)], BF16)
            onescol = singles.tile([128, T // 128, 1], F32)
            nc.vector.memset(onescol, 1.0)
            nc.vector.tensor_copy(v_sb[:, :, C:C + 1], onescol)
            nc.vector.tensor_copy(v_sb[:, :, 2 * C + 1:2 * C + 2], onescol)

            # strided view of qT for phase-2 blocks: [128, g, blk, 128]
            qT_v = qT_sb[:, :].rearrange("p (g k c) -> p g k c", g=8, c=128)

            # ================= phase 1: projections + norm + rope =========
            st = {}

            def emit_proj(ch):
                t0 = ch * TCH
                xt = xtp.tile([128, 8, TCH], BF16, tag="xt")
                nc.sync.dma_start(out=xt, in_=xT4[:, :, t0:t0 + TCH])
                raq = ropep.tile([128, TCH], BF16, tag="raq")
                rbq = ropep.tile([128, TCH], BF16, tag="rbq")
                rak = ropep.tile([128, TCH], BF16, tag="rak")
                rbk = ropep.tile([128, TCH], BF16, tag="rbk")
                nc.sync.dma_start(out=raq, in_=ropeAq[:, t0:t0 + TCH])
                nc.sync.dma_start(out=rbq, in_=ropeBq[:, t0:t0 + TCH])
                nc.sync.dma_start(out=rak, in_=ropeAk[:, t0:t0 + TCH])
                nc.sync.dma_start(out=rbk, in_=ropeBk[:, t0:t0 + TCH])

                psqk = ps_big.tile([128, 2 * TCH], F32, tag="mm2",
                                   name=f"psqk_{ch}")
                for a in range(8):
                    nc.tensor.matmul(psqk[:, 0:TCH], wq_sb[:, a, :],
                                     xt[:, a, :], start=(a == 0), stop=(a == 7))
                for a in range(8):
                    nc.tensor.matmul(psqk[:, TCH:2 * TCH], wk_sb[:, a, :],
                                     xt[:, a, :], start=(a == 0), stop=(a == 7))
                # stage raw q/k in SBUF bf16 (engines read one PSUM input)
                qraw = workp.tile([128, TCH], BF16, tag="qraw", bufs=3)
                nc.scalar.copy(qraw, psqk[:, 0:TCH])
                kraw = workp.tile([128, TCH], BF16, tag="kraw", bufs=3)
                nc.scalar.copy(kraw, psqk[:, TCH:2 * TCH])
                sq = workp.tile([128, TCH], F32R, tag="sq", bufs=2)
                nc.vector.tensor_mul(sq, qraw, qraw)
                sqk = workp.tile([128, TCH], F32R, tag="sqk", bufs=2)
                nc.vector.tensor_mul(sqk, kraw, kraw)
                st[ch] = dict(psqk=psqk, sq=sq, sqk=sqk, qraw=qraw,
                              kraw=kraw, raq=raq, rbq=rbq, rak=rak, rbk=rbk,
                              xt=xt)

            def emit_projv(ch):
                s = st[ch]
                psv = ps_op.tile([128, TCH], F32, tag="po", name=f"psv_{ch}")
                for a in range(8):
                    nc.tensor.matmul(psv, wv_sb[:, a, :], s["xt"][:, a, :],
                                     start=(a == 0), stop=(a == 7))
                s["psv"] = psv

            def emit_norm(ch):
                s = st[ch]
                # v copy first so psv's PSUM slot frees promptly
                vt = workp.tile([128, TCH], BF16, tag="vt", bufs=3)
                nc.scalar.copy(vt, s["psv"])
                s["vt"] = vt
                msq = ps_op.tile([2, TCH], F32, tag="po", name=f"msq_{ch}")
                nc.tensor.matmul(msq, ones2, s["sq"], start=True, stop=True)
                msk = ps_op.tile([2, TCH], F32, tag="po", name=f"msk_{ch}")
                nc.tensor.matmul(msk, ones2, s["sqk"], start=True, stop=True)
                # rstd = sqrt(1/(ms/C)): fast-approx recip on DVE reads
                # ms straight from PSUM (1/C folded into ones2; eps dropped -
                # it only guards all-zero rows), then ACT Sqrt writes bf16
                rqi = workp.tile([2, TCH], F32, tag="rqi", bufs=2,
                                 name=f"rqi_{ch}")
                nc.vector.reciprocal_approx_fast(rqi, msq)
                rki = workp.tile([2, TCH], F32, tag="rki", bufs=2,
                                 name=f"rki_{ch}")
                nc.vector.reciprocal_approx_fast(rki, msk)
                nc.scalar.activation(rstd_s[0:2, ch % 2, 0:TCH], rqi,
                                     AF.Sqrt, bias=0.0, scale=1.0)
                nc.scalar.activation(rstd_s[0:2, ch % 2, TCH:2 * TCH], rki,
                                     AF.Sqrt, bias=0.0, scale=1.0)

            def emit_bcast(ch):
                s = st[ch]
                bcq_ps = ps_op.tile([128, TCH], F32, tag="po",
                                    name=f"bcq_{ch}")
                nc.tensor.matmul(bcq_ps, bcq, rstd_s[:, ch % 2, 0:TCH],
                                 start=True, stop=True)
                bck_ps = ps_op.tile([128, TCH], F32, tag="po",
                                    name=f"bck_{ch}")
                nc.tensor.matmul(bck_ps, bck, rstd_s[:, ch % 2, TCH:2 * TCH],
                                 start=True, stop=True)
                s["bcq_ps"], s["bck_ps"] = bcq_ps, bck_ps

            def emit_rope(ch):
                t0 = ch * TCH
                s = st.pop(ch)
                rotq = workp.tile([128, TCH], BF16, tag="rotq")
                rotk = workp.tile([128, TCH], BF16, tag="rotk")
                qraw, kraw = s["qraw"], s["kraw"]
                for g0 in (0, 64):
                    nc.sync.dma_start(out=rotq[g0:g0 + 32, :],
                                      in_=qraw[g0 + 32:g0 + 64, :])
                    nc.sync.dma_start(out=rotq[g0 + 32:g0 + 64, :],
                                      in_=qraw[g0:g0 + 32, :])
                    nc.sync.dma_start(out=rotk[g0:g0 + 32, :],
                                      in_=kraw[g0 + 32:g0 + 64, :])
                    nc.sync.dma_start(out=rotk[g0 + 32:g0 + 64, :],
                                      in_=kraw[g0:g0 + 32, :])
                t1q = workp.tile([128, TCH], BF16, tag="t1q")
                nc.vector.tensor_mul(t1q, s["raq"], qraw)
                t2q = workp.tile([128, TCH], BF16, tag="t2q")
                nc.gpsimd.tensor_mul(t2q, s["rbq"], rotq)
                qfu = workp.tile([128, TCH], BF16, tag="qfu")
                nc.vector.tensor_add(qfu, t1q, t2q)
                nc.vector.tensor_mul(qT_sb[:, t0:t0 + TCH], qfu, s["bcq_ps"])
                t1k = workp.tile([128, TCH], BF16, tag="t1k")
                nc.vector.tensor_mul(t1k, s["rak"], kraw)
                t2k = workp.tile([128, TCH], BF16, tag="t2k")
                nc.gpsimd.tensor_mul(t2k, s["rbk"], rotk)
                kf = workp.tile([128, TCH], BF16, tag="kf")
                nc.vector.tensor_add(kf, t1k, t2k)
                nc.vector.tensor_mul(kTz0[0:64, t0:t0 + TCH], kf[0:64, :],
                                     s["bck_ps"][0:64, :])
                nc.vector.tensor_mul(kTz1[64:128, t0:t0 + TCH],
                                     kf[64:128, :], s["bck_ps"][64:128, :])
                # v: bf16 transpose into v_sb
                pt = ps_op.tile([128, TCH // 128, 128], BF16, tag="po",
                                name=f"pt_{ch}")
                for s5 in range(TCH // 128):
                    nc.tensor.transpose(pt[:, s5, :],
                                        s["vt"][:, s5 * 128:(s5 + 1) * 128],
                                        identb)
                for s5 in range(TCH // 128):
                    tt = (t0 + s5 * 128) // 128
                    nc.vector.tensor_copy(v_sb[:, tt, 0:C], pt[:, s5, 0:C])
                    nc.vector.tensor_copy(v_sb[:, tt, C + 1:2 * C + 1],
                                          pt[:, s5, C:2 * C])

            emit_proj(0)
            emit_projv(0)
            emit_norm(0)
            emit_proj(1)
            emit_projv(1)
            for ch in range(2, NCHUNK):
                emit_norm(ch - 1)
                emit_proj(ch)
                emit_bcast(ch - 2)
                emit_projv(ch)
                emit_rope(ch - 2)
            emit_norm(NCHUNK - 1)
            for ch in (NCHUNK - 2, NCHUNK - 1):
                emit_bcast(ch)
                emit_rope(ch)

            wo_sb = singles.tile([128, 8, D], BF16)
            nc.sync.dma_start(out=wo_sb, in_=wo4)

            # ================= phase 2: attention + pipelined A2A =========
            bounce_in = [dram.tile([NCORES * 128, 128], BF16,
                                   name=f"bnc_in_{k}") for k in range(NBLK)]
            bounce_out = [dram.tile([NCORES * 128, 128], BF16,
                                    name=f"bnc_out_{k}") for k in range(NBLK)]

            def emit_batch_compute(blk, b):
                """scores+exp+av for (block, batch); stage po/den in SBUF"""
                pos = [ps_op.tile([C + 1, TCH], F32, tag="po",
                                  name=f"pos_{blk}_{b}_{lh}")
                       for lh in range(LH)]
                qv = qT_v[:, 4 * b:4 * b + 4, blk, :]
                pend = []

                def emit_scores(jt):
                    j0 = b * S + jt * 128
                    pss = ps_big.tile([128, 2 * TCH], F32, tag="mm2",
                                      name=f"pss_{blk}_{b}_{jt}")
                    nc.tensor.matmul(pss[:, 0:TCH], kTz0[:, j0:j0 + 128], qv,
                                     start=True, stop=True)
                    nc.tensor.matmul(pss[:, TCH:2 * TCH], kTz1[:, j0:j0 + 128],
                                     qv, start=True, stop=True)
                    ex = expp.tile([128, 2 * TCH], BF16, tag="ex",
                                   name=f"ex_{blk}_{b}_{jt}")
                    nc.scalar.activation(ex, pss, AF.Exp, bias=0.0, scale=1.0)
                    return ex

                def emit_av(jt, ex):
                    j0 = b * S + jt * 128
                    for lh in range(LH):
                        nc.tensor.matmul(
                            pos[lh],
                            v_sb[:, j0 // 128,
                                 lh * (C + 1):(lh + 1) * (C + 1)],
                            ex[:, lh * TCH:(lh + 1) * TCH],
                            start=(jt == 0), stop=(jt == S // 128 - 1))

                for jt in range(S // 128):
                    pend.append((jt, emit_scores(jt)))
                    if len(pend) > 2:
                        pj, pex = pend.pop(0)
                        emit_av(pj, pex)
                while pend:
                    pj, pex = pend.pop(0)
                    emit_av(pj, pex)

                # stage po in SBUF (frees the pos PSUM slots) + 1/den
                po_sb = [workp.tile([C + 1, TCH], F32, tag=f"posb{lh}",
                                    bufs=2, name=f"posb_{blk}_{b}_{lh}")
                         for lh in range(LH)]
                for lh in range(LH):
                    nc.vector.tensor_copy(po_sb[lh], pos[lh])
                den2 = workp.tile([2, TCH], F32, tag="den2", bufs=2,
                                  name=f"den2_{blk}_{b}")
                for lh in range(LH):
                    nc.gpsimd.dma_start(out=den2[lh:lh + 1, :],
                                        in_=po_sb[lh][C:C + 1, :])
                rdf = workp.tile([2, TCH], F32, tag="rdf", bufs=2,
                                 name=f"rdf_{blk}_{b}")
                nc.vector.reciprocal_approx_fast(rdf, den2)
                # broadcast 1/den to 64 partitions via a DRAM round trip
                # (keeps the whole normalize chain off the PE queue)
                rdf_dr = dram.tile([2, TCH], F32, tag="rdf_dr", bufs=2,
                                   name=f"rdfdr_{blk}_{b}")
                nc.gpsimd.dma_start(out=rdf_dr, in_=rdf)
                atts = []
                for lh in range(LH):
                    nrm_sb = workp.tile([C, TCH], F32, tag=f"nrmsb{lh}",
                                        bufs=2, name=f"nrmsb_{blk}_{b}_{lh}")
                    nc.gpsimd.dma_start(
                        out=nrm_sb,
                        in_=bass.AP(tensor=rdf_dr.tensor, offset=rdf_dr.offset
                                    + lh * TCH, ap=[[0, C], [1, TCH]]))
                    att = workp.tile([C, TCH], BF16, tag=f"att{lh}", bufs=2,
                                     name=f"att_{blk}_{b}_{lh}")
                    nc.vector.tensor_mul(att, po_sb[lh][0:C, :], nrm_sb)
                    atts.append(att)
                # scatter: columns are 4 groups x 128 tokens (batch b)
                for gg in range(4):
                    g = 4 * b + gg
                    for lh in range(LH):
                        nc.gpsimd.dma_start(
                            out=bounce_in[blk][g * 128 + lh * C:
                                               g * 128 + (lh + 1) * C, :],
                            in_=atts[lh][:, gg * 128:(gg + 1) * 128])

            def emit_block(blk):
                emit_batch_compute(blk, 0)
                emit_batch_compute(blk, 1)
                nc.gpsimd.collective_compute(
                    "AllToAll", mybir.AluOpType.bypass,
                    replica_groups=[list(range(NCORES))],
                    ins=[bounce_in[blk][:, :].opt()],
                    outs=[bounce_out[blk][:, :].opt()])

            def emit_outproj(blk):
                att_all = outp.tile([128, 8, 128], BF16, tag="aat")
                nc.sync.dma_start(
                    out=att_all,
                    in_=bounce_out[blk][:, :].rearrange("(a p) t -> p a t",
                                                        p=128))
                pp = ps_big.tile([128, D], F32, tag="mm2", name=f"pp_{blk}")
                for nh in range(2):
                    for a in range(8):
                        nc.tensor.matmul(
                            pp[:, nh * 512:(nh + 1) * 512], att_all[:, a, :],
                            wo_sb[:, a, nh * 512:(nh + 1) * 512],
                            start=(a == 0), stop=(a == 7))
                out_sb = outp.tile([128, D], F32, tag="osb")
                nc.vector.tensor_copy(out_sb, pp)
                nc.sync.dma_start(out=out[blk * 128:(blk + 1) * 128, :],
                                  in_=out_sb)

            emit_block(0)
            emit_block(1)
            emit_block(2)
            emit_outproj(0)
            emit_block(3)
            emit_outproj(1)
            emit_outproj(2)
            emit_outproj(3)

    nc.compile()
    return nc


def kernel(x, rope_emb, Wq, Wk, Wv, q_norm_w, k_norm_w, Wout):
    global LAST_RESULTS
    if "nc" not in _CACHE:
        _CACHE["nc"] = _build()
    nc = _CACHE["nc"]

    bf = ml_dtypes.bfloat16
    # batch-major tokens: t = b*S + s
    x2 = np.ascontiguousarray(
        np.transpose(np.asarray(x, np.float32), (1, 0, 2)).reshape(T, D))
    xT_np = np.ascontiguousarray(x2.T).astype(bf)

    re = np.asarray(rope_emb, np.float32)
    cosT = np.ascontiguousarray(re[:, :, 0, 0].T)    # [32, S]
    r01T = np.ascontiguousarray(re[:, :, 0, 1].T)
    r10T = np.ascontiguousarray(re[:, :, 1, 0].T)
    cos2 = np.concatenate([cosT, cosT], axis=1)      # [32, T] batch-major
    r01_2 = np.concatenate([r01T, r01T], axis=1)
    r10_2 = np.concatenate([r10T, r10T], axis=1)
    ropeA = np.concatenate([cos2, cos2, cos2, cos2], axis=0)   # [128, T]
    ropeB = np.concatenate([r01_2, r10_2, r01_2, r10_2], axis=0)

    qw_np = np.asarray(q_norm_w, np.float32)
    kw_np = np.asarray(k_norm_w, np.float32)
    wpat_q = np.concatenate([qw_np, qw_np])          # [128]
    wpat_k = np.concatenate([kw_np, kw_np])
    # partner row: swap 32-blocks within each 64-row head group
    pidx = np.arange(128)
    pidx = np.where((pidx % 64) < 32, pidx + 32, pidx - 32)
    ropeAq_np = (ropeA * wpat_q[:, None]).astype(bf)
    ropeBq_np = (ropeB * wpat_q[pidx][:, None]).astype(bf)
    ropeAk_np = (ropeA * wpat_k[:, None]).astype(bf)
    ropeBk_np = (ropeB * wpat_k[pidx][:, None]).astype(bf)

    Wq = np.asarray(Wq, np.float32)
    Wk = np.asarray(Wk, np.float32)
    Wv = np.asarray(Wv, np.float32)
    Wout = np.ascontiguousarray(np.asarray(Wout, np.float32)).astype(bf)

    bcw_np = np.zeros((128, 128), np.float32)
    bcw_np[0, 0:64] = 1.0                      # bcq
    bcw_np[1, 64:128] = 1.0
    bcw_np[32, 0:64] = 1.0 / np.sqrt(C)        # bck
    bcw_np[33, 64:128] = 1.0 / np.sqrt(C)
    bcw_np[64, 0:64] = 1.0                     # bcd0 row0
    bcw_np[97, 0:64] = 1.0                     # bcd1 row1

    in_maps = []
    for g in range(NCORES):
        sl = slice(g * LC, (g + 1) * LC)
        in_maps.append({
            "xT": xT_np,
            "wq": np.ascontiguousarray(Wq[:, sl]).astype(bf),
            "wk": np.ascontiguousarray(Wk[:, sl]).astype(bf),
            "wv": np.ascontiguousarray(Wv[:, sl]).astype(bf),
            "wo": Wout,
            "ropeAq": ropeAq_np, "ropeBq": ropeBq_np,
            "ropeAk": ropeAk_np, "ropeBk": ropeBk_np,
            "bcw": bcw_np,
        })

    res = run_bass_kernel_spmd(nc, in_maps, core_ids=list(range(NCORES)))
    LAST_RESULTS = res
    # core g returns out rows = its 4 x 128-token tiles: token t = g*512 + r
    out_full = np.concatenate([res.results[g]["out"] for g in range(NCORES)],
                              axis=0)                 # [T, D] batch-major
    return np.ascontiguousarray(
        out_full.reshape(B, S, D).transpose(1, 0, 2).astype(np.float32))
